# revision 53
# baseline (speedup 1.0000x reference)
"""Distributed GQA attention kernel for 8 TRN2 NeuronCores (v2).

Problem: B=2, S=2048, D=2048, 32 q-heads / 8 kv-heads, hd=64, causal + RoPE.

Strategy (kv-head tensor parallel, zero collectives):
  - Core c owns kv-head c (q-heads 4c..4c+3) for BOTH batches over ALL rows.
    Every core loads the full x (host-pretransposed to xT bf16) and projects
    Q (4 heads), K, V (1 kv head each) for all 4096 rows. K/V never leave the
    core, so there are NO collectives. Each core computes a PARTIAL output
    (its 4 heads x its 256 wo rows) and the host sums the 8 partials.
  - Attention runs "transposed": scoresT = ksl.T @ qT4 with keys on
    partitions; softmax without max-subtraction; denominator via a ones
    column appended to the V stationary (padded to M=128 for FWL).
  - v2 changes vs v1:
      * scalar engine runs ONLY exp (all copies/DMA queueing moved off it)
      * PV stationary padded to M=128 (FWL) + kb-pairs merged to N=1024
      * softmax norm: reciprocal straight from PSUM + 2 strided muls
        writing attnT directly (no pvs/rsum copies, no 4 small muls)
      * fine-grained software pipeline: proj/out-proj matmuls are emitted
        as "filler" between attention iterations so PE never waits on exp
      * x loaded as [128,1024] double-chunk tiles (halves DMA queue ops)
      * crep/ssig loaded small ([32,2048]/[64,2048]) and replicated on-chip
      * V transposed with ONE dma_start_transpose per chunk (64-col blocks)
      * out written as [128,1024] tiles (halves out DMA queue ops)

kernel(**inputs) -> np.ndarray  takes full inputs, returns full [2,2048,2048].
"""

import functools
import os
import sys
import types
from collections import deque

import numpy as np
import ml_dtypes

BF16 = ml_dtypes.bfloat16

B, S, D = 2, 2048, 2048
NH, NKV, HD = 32, 8, 64
BS = B * S               # 4096 rows total (b-major)
NCORES = 8


# --------------------------------------------------------------------------
# device graph (identical on all cores; per-core weights via input data)
# --------------------------------------------------------------------------

@functools.lru_cache(maxsize=None)
def _build_nc():
    import concourse.bacc as bacc
    import concourse.mybir as mybir
    import concourse.tile as tile

    BF = mybir.dt.bfloat16
    F32 = mybir.dt.float32
    EXP = mybir.ActivationFunctionType.Exp

    nc = bacc.Bacc(trn_type="TRN2", target_bir_lowering=False, debug=False,
                   num_devices=NCORES)

    dbg = bool(int(os.environ.get("KERNEL_DEBUG", "0")))
    if dbg:
        kT_dbg = nc.declare_dram_parameter("kT_dbg", [64, BS], BF,
                                           isOutput=True)
        vO_dbg = nc.declare_dram_parameter("vO_dbg", [128, 32 * 128], BF,
                                           isOutput=True)
        qT_dbg = nc.declare_dram_parameter("qT_dbg", [64, 32 * 512], BF,
                                           isOutput=True)
        aT_dbg = nc.declare_dram_parameter("aT_dbg", [128, 2 * BS], BF,
                                           isOutput=True)

    xT_d = nc.declare_dram_parameter("xT", [D, BS], BF, isOutput=False)
    wq_d = nc.declare_dram_parameter("wq", [2, 128, 2048], BF, isOutput=False)
    wkv_d = nc.declare_dram_parameter("wkv", [128, 2048], BF, isOutput=False)
    wo_d = nc.declare_dram_parameter("wo", [2, 128, D], BF, isOutput=False)
    crep_d = nc.declare_dram_parameter("crep", [32, S], BF, isOutput=False)
    ssig_d = nc.declare_dram_parameter("ssig", [64, S], BF, isOutput=False)
    dmask_d = nc.declare_dram_parameter("dmask", [128, 512], BF, isOutput=False)
    out_d = nc.declare_dram_parameter("out", [BS, D], BF, isOutput=True)

    with tile.TileContext(nc) as tc:
        with tc.tile_pool(name="const", bufs=1) as cpool, \
             tc.tile_pool(name="persist", bufs=1) as ppool, \
             tc.tile_pool(name="xsmall", bufs=16) as xspool, \
             tc.tile_pool(name="xbig", bufs=18) as xpool, \
             tc.tile_pool(name="work", bufs=2) as tpool, \
             tc.tile_pool(name="attn", bufs=2) as apool, \
             tc.tile_pool(name="ps", bufs=1, space="PSUM") as pspool:

            # ---- constants: weights first (they gate first projections) ----
            # wkv/wq0 on gpsimd so the sync queue leads with x tile 0
            wkv_sb = cpool.tile([128, 2048], BF, name="wkv_sb", tag="wkv_sb")
            nc.gpsimd.dma_start(out=wkv_sb[:, :], in_=wkv_d[:, :])
            wq_sb = []
            for hp in range(2):
                w_ = cpool.tile([128, 2048], BF, name=f"wq_sb{hp}",
                                tag=f"wq_sb{hp}")
                eng = nc.gpsimd if hp == 0 else nc.scalar
                eng.dma_start(out=w_[:, :], in_=wq_d[hp, :, :])
                wq_sb.append(w_)

            # x loaded as [128, 1024] per-ktile tiles, 2 chunks per group
            # (2KB per-partition DMA lines: per-line overhead ~28ns makes
            # 1KB-line transfers cap at ~35GB/s/queue; 2KB lines ~60GB/s).
            # Queues carry ONLY non-blocking bulk loads to avoid
            # head-of-line stalls behind data-dependent DMAs.
            xg_tiles = {}
            x_state = {"issued": 0, "freed": 0}
            XBUFS = 30

            def x_pump():
                # keep x streaming continuously, up to XBUFS tiles ahead of
                # the consumption frontier (issues alternate sync/scalar,
                # both queues carry only non-blocking bulk loads)
                while (x_state["issued"] < 64
                       and x_state["issued"] < x_state["freed"] + XBUFS):
                    i = x_state["issued"]
                    g, k = divmod(i, 16)
                    t_ = xpool.tile([128, 1024], BF, name="xg", tag="xg",
                                    bufs=XBUFS)
                    eng = nc.sync if i % 2 == 0 else nc.scalar
                    eng.dma_start(out=t_[:, :],
                                  in_=xT_d[k * 128:(k + 1) * 128,
                                           g * 1024:(g + 1) * 1024])
                    xg_tiles.setdefault(g, [None] * 16)[k] = t_
                    x_state["issued"] += 1

            def issue_group0():
                # group 0 loaded as half-tile DMAs, ALL chunk-0 halves
                # first across 3 queues: chunk 0 gates the entire attention
                # pipeline start, so its 2.1MB must not wait on chunk-1
                # bytes (gpsimd is free until the first rope ~15us in)
                lst = xg_tiles.setdefault(0, [None] * 16)
                for k in range(16):
                    lst[k] = xpool.tile([128, 1024], BF, name="xg",
                                        tag="xg", bufs=XBUFS)
                for half in range(2):
                    for k in range(16):
                        eng = (nc.sync, nc.scalar, nc.gpsimd)[k % 3]
                        eng.dma_start(
                            out=lst[k][:, half * 512:(half + 1) * 512],
                            in_=xT_d[k * 128:(k + 1) * 128,
                                     half * 512:half * 512 + 512])
                x_state["issued"] = 16

            # small const loads on the scalar queue (idle before first exp)
            crep = cpool.tile([128, BS], BF, name="crep", tag="crep")
            nc.scalar.dma_start(out=crep[0:32, 0:S], in_=crep_d[:, :])
            ssig = cpool.tile([128, BS], BF, name="ssig", tag="ssig")
            nc.scalar.dma_start(out=ssig[0:64, 0:S], in_=ssig_d[:, :])
            dmask = cpool.tile([128, 512], BF, name="dmask", tag="dmask")
            nc.scalar.dma_start(out=dmask[:, :], in_=dmask_d[:, :])

            # startup x: group 0 chunk-first, then pump ahead
            issue_group0()
            x_pump()
            # replicate crep rows 0:32 -> 0:128, ssig rows 0:64 -> 0:128,
            # then both across the batch column halves (SBUF->SBUF DMAs,
            # gpsimd queue so the sync queue stays pure bulk-load)
            nc.gpsimd.dma_start(out=crep[32:64, 0:S], in_=crep[0:32, 0:S])
            nc.gpsimd.dma_start(out=crep[64:128, 0:S], in_=crep[0:64, 0:S])
            nc.gpsimd.dma_start(out=ssig[64:128, 0:S], in_=ssig[0:64, 0:S])
            nc.gpsimd.dma_start(out=crep[:, S:BS], in_=crep[:, 0:S])
            nc.gpsimd.dma_start(out=ssig[:, S:BS], in_=ssig[:, 0:S])

            wot = []
            for t in range(2):
                w_ = cpool.tile([128, D], BF, name=f"wot{t}", tag=f"wot{t}")
                wot.append(w_)

            # shifted identity for TensorE V-transpose: rows 64:128 hold
            # I_64 so lhsT/rhs share base_partition 64 (row group (64,0))
            ident = cpool.tile([128, 64], BF, name="ident", tag="ident")
            nc.gpsimd.memset(ident[:, :], 0.0)
            nc.gpsimd.affine_select(
                out=ident[:, :], in_=ident[:, :],
                compare_op=mybir.AluOpType.not_equal, fill=1.0,
                base=-64, pattern=[[-1, 64]], channel_multiplier=1)

            # ---- persistent activations ----
            # kT rows 0:64 data, 64:128 DMA-duplicated copy so K=64 score
            # matmuls can row-pack two q-blocks per PE pass.
            kT = ppool.tile([128, BS], BF, name="kT", tag="kT")
            # vOnes: 32 blocks of 128 cols: [v 64 | ones 1 | zeros 63].
            # M=128 stationary (vs 65) enables fast weight load.
            vOnes = ppool.tile([128, 32 * 128], BF, name="vOnes", tag="vOnes")
            vv = vOnes.rearrange("p (n w) -> p n w", w=128)
            nc.vector.memset(vv[:, :, 64:128], 0.0)
            nc.vector.memset(vv[:, :, 64:65], 1.0)
            # qT4 cols: (b*16 + qb)*512 + h*128 + r   (h = head 0..3 local)
            qT4 = ppool.tile([128, 32 * 512], BF, name="qT4", tag="qT4")
            qv = qT4[0:64, :].rearrange("p (blk h r) -> p blk h r", h=4, r=128)
            # attnT: [128 = (2h, hd), t*4096 + rb*128 + q]
            attnT = ppool.tile([128, 2 * BS], BF, name="attnT", tag="attnT")
            atv = attnT.rearrange("p (t q) -> p t q", t=2)

            # ---- projection work units (emitted as 4-MM packets) ----
            pstate = {}

            def xsrc(c):
                lst = xg_tiles[c // 2]
                half = c % 2
                return lambda k: lst[k][:, half * 512:(half + 1) * 512]

            def kv_tail(c, ps):
                cs = c * 512
                kvraw = tpool.tile([128, 512], BF, name="kvraw", tag="kvraw",
                                   bufs=3)
                nc.vector.tensor_copy(out=kvraw[:, :], in_=ps[:, :])
                # K rope (rows 0:64; [ev32|od32] de-interleaved). The
                # rotate-half term reads kvraw with a 32-row partition shift
                # (inputs pairwise aligned; out base may differ) — ssig is
                # host-prepped with the matching sign pattern [+s, -s], so
                # no shuffle DMAs are needed.
                rot = tpool.tile([64, 512], BF, name="rot", tag="rot")
                nc.vector.tensor_mul(rot[0:32, :], kvraw[32:64, :],
                                     ssig[32:64, cs:cs + 512])
                nc.vector.tensor_mul(rot[32:64, :], kvraw[0:32, :],
                                     ssig[0:32, cs:cs + 512])
                nc.vector.tensor_mul(kT[0:64, cs:cs + 512], kvraw[0:64, :],
                                     crep[0:64, cs:cs + 512])
                nc.vector.tensor_add(kT[0:64, cs:cs + 512],
                                     kT[0:64, cs:cs + 512], rot[0:64, :])
                nc.gpsimd.dma_start(out=kT[64:128, cs:cs + 512],
                                    in_=kT[0:64, cs:cs + 512])
                # V natural layout via TensorE transposes (frees the DMA
                # queues of data-dependent transposes). pst shares the pj
                # psum rotation; its bf16 view holds 4 transposed blocks.
                pst = pspool.tile([128, 512], F32, name="pst", tag="pj",
                                  bufs=2)
                pst_bf = pst.bitcast(BF)
                for j in range(4):
                    nc.tensor.transpose(
                        pst_bf[:, j * 64:(j + 1) * 64],
                        kvraw[64:128, j * 128:(j + 1) * 128],
                        ident[64:128, :])
                nc.vector.tensor_copy(
                    out=vv[:, c * 4:(c + 1) * 4, 0:64],
                    in_=pst_bf[:, 0:256].rearrange("p (j w) -> p j w", w=64))

            def q_tail(c, hp, psq):
                cs = c * 512
                qraw = tpool.tile([128, 512], BF, name="qraw", tag="qraw")
                nc.vector.tensor_copy(out=qraw[:, :], in_=psq[:, :])
                rotq = tpool.tile([128, 512], BF, name="rotq", tag="rotq")
                for (db, sb) in ((0, 32), (32, 0), (64, 96), (96, 64)):
                    nc.vector.tensor_mul(rotq[db:db + 32, :],
                                         qraw[sb:sb + 32, :],
                                         ssig[sb:sb + 32, cs:cs + 512])
                q2 = tpool.tile([128, 512], BF, name="q2", tag="q2")
                nc.vector.tensor_mul(q2[:, :], qraw[:, :],
                                     crep[:, cs:cs + 512])
                for ph in range(2):
                    h = 2 * hp + ph
                    dst = qv[:, 4 * c:4 * c + 4, h, :]
                    s2 = q2[ph * 64:(ph + 1) * 64, :].rearrange(
                        "p (j r) -> p j r", r=128)
                    s3 = rotq[ph * 64:(ph + 1) * 64, :].rearrange(
                        "p (j r) -> p j r", r=128)
                    nc.vector.tensor_add(dst, s2, s3)
                if hp == 1:
                    qc0 = 2048 * c
                    nc.gpsimd.dma_start(out=qT4[64:128, qc0:qc0 + 2048],
                                        in_=qT4[0:64, qc0:qc0 + 2048])

            def kv_packet(c, q):
                if q == 0:
                    pstate[("kv", c)] = pspool.tile(
                        [128, 512], F32, name="pskv", tag="pj", bufs=2)
                ps = pstate[("kv", c)]
                xs = xsrc(c)
                for k in range(4 * q, 4 * q + 4):
                    nc.tensor.matmul(ps[:, :],
                                     lhsT=wkv_sb[:, k * 128:(k + 1) * 128],
                                     rhs=xs(k), start=(k == 0), stop=(k == 15))
                if q == 3:
                    kv_tail(c, ps)

            def q_packet(c, hp, q):
                key = ("q", c, hp)
                if q == 0:
                    pstate[key] = pspool.tile(
                        [128, 512], F32, name="psq", tag="pj", bufs=2)
                psq = pstate[key]
                xs = xsrc(c)
                for k in range(4 * q, 4 * q + 4):
                    nc.tensor.matmul(psq[:, :],
                                     lhsT=wq_sb[hp][:, k * 128:(k + 1) * 128],
                                     rhs=xs(k), start=(k == 0), stop=(k == 15))
                # this packet was the last pass over its x tiles: advance
                # the x frontier and pump the stream
                if hp == 1 and c % 2 == 1:
                    x_state["freed"] += 4
                    x_pump()
                if q == 3:
                    q_tail(c, hp, psq)

            # ---- out-projection units ----
            obstate = {}

            def op_unit(rb, dc):
                po = pspool.tile([128, 512], F32, name="po", tag="pj", bufs=2)
                for t in range(2):
                    nc.tensor.matmul(
                        po[:, :],
                        lhsT=attnT[:, t * BS + rb * 128:
                                   t * BS + (rb + 1) * 128],
                        rhs=wot[t][:, dc * 512:(dc + 1) * 512],
                        start=(t == 0), stop=(t == 1))
                if dc == 0:
                    obstate[rb] = tpool.tile([128, 2048], BF, name="ob",
                                             tag="ob", bufs=3)
                ob = obstate[rb]
                nc.vector.tensor_copy(out=ob[:, dc * 512:(dc + 1) * 512],
                                      in_=po[:, :])
                if rb >= 28:
                    # tail rbs: write each quarter as soon as its cast
                    # lands, rotating queues, so the final writes overlap
                    # the remaining compute instead of serializing the end
                    eng = (nc.sync, nc.scalar, nc.gpsimd, nc.sync)[dc]
                    eng.dma_start(
                        out=out_d[rb * 128:(rb + 1) * 128,
                                  dc * 512:(dc + 1) * 512],
                        in_=ob[:, dc * 512:(dc + 1) * 512])
                elif dc == 3:
                    eng = (nc.sync, nc.gpsimd, nc.scalar)[rb % 3]
                    eng.dma_start(
                        out=out_d[rb * 128:(rb + 1) * 128, :],
                        in_=ob[:, :])

            # ---- attention ----
            def emit_norm(b, qb, pvt, j):
                base = j * 512
                col = (b * 16 + qb) * 128
                # NOTE: reciprocal_approx_fast (custom DVE) silently misreads
                # PSUM sources — the denominator row MUST bounce through SBUF.
                rsum = apool.tile([1, 512], F32, name="rsum", tag="rsum",
                                  bufs=1)
                nc.vector.tensor_copy(out=rsum[0:1, :],
                                      in_=pvt[64:65, base:base + 512])
                rinv = apool.tile([1, 512], F32, name="rinv", tag="rinv",
                                  bufs=1)
                nc.vector.reciprocal_approx_fast(out=rinv[0:1, :],
                                                 in_=rsum[0:1, :])
                rb64 = apool.tile([64, 512], F32, name="rb64", tag="rb64")
                nc.gpsimd.partition_broadcast(rb64[:, :], rinv[0:1, :])
                srcv = pvt[0:64, base:base + 512].rearrange(
                    "p (h q) -> p h q", q=128)
                rbv = rb64.rearrange("p (h q) -> p h q", q=128)
                for ph in range(2):
                    nc.vector.tensor_mul(
                        atv[ph * 64:(ph + 1) * 64, :, col:col + 128],
                        srcv[:, ph::2, :], rbv[:, ph::2, :])
                push_ops(b, qb)

            def emit_pv(b, qb0, kb, active, pr, pvt):
                bb = b * 16 + kb
                lhs = vOnes[:, bb * 128:(bb + 1) * 128]
                for jj, qb in enumerate(active):
                    j = qb - qb0
                    nc.tensor.matmul(
                        pvt[:, j * 512:(j + 1) * 512], lhsT=lhs,
                        rhs=pr[:, jj * 512:(jj + 1) * 512],
                        start=(kb == 0), stop=(kb == qb))
                    if kb == qb:
                        emit_norm(b, qb, pvt, j)

            def attn_iter(b, m, kb, pvt, pend, depth=3):
                qb0 = 2 * m
                active = [qb for qb in (qb0, qb0 + 1) if qb >= kb]
                ks = b * S + kb * 128
                qs0 = (b * 16 + qb0) * 512
                sct = pspool.tile([128, 1024], F32, name="sc", tag="sc",
                                  bufs=2)
                if len(active) == 2:
                    nc.tensor.matmul(sct[:, 0:512],
                                     lhsT=kT[0:64, ks:ks + 128],
                                     rhs=qT4[0:64, qs0:qs0 + 512],
                                     start=True, stop=True,
                                     tile_position=(0, 0))
                    nc.tensor.matmul(sct[:, 512:1024],
                                     lhsT=kT[64:128, ks:ks + 128],
                                     rhs=qT4[64:128, qs0 + 512:qs0 + 1024],
                                     start=True, stop=True,
                                     tile_position=(64, 0))
                else:
                    qs1 = qs0 + 512
                    nc.tensor.matmul(sct[:, 0:512],
                                     lhsT=kT[0:64, ks:ks + 128],
                                     rhs=qT4[0:64, qs1:qs1 + 512],
                                     start=True, stop=True,
                                     tile_position=(0, 0))
                w = 512 * len(active)
                pr = apool.tile([128, 1024], BF, name="probs", tag="probs",
                                bufs=5)
                nc.scalar.activation(out=pr[:, 0:w], in_=sct[:, 0:w],
                                     func=EXP, scale=0.125)
                if active[0] == kb:
                    nc.gpsimd.tensor_mul(pr[:, 0:512], pr[:, 0:512],
                                         dmask[:, :])
                pend.append((kb, active, pr))
                if len(pend) > depth:
                    pk, pa, ppr = pend.pop(0)
                    emit_pv(b, qb0, pk, pa, ppr, pvt)

            # ---- filler scheduler ----
            filler_proj = deque()
            for c in range(8):
                for q in range(4):
                    filler_proj.append((c, ("kv", c, q)))
                for hp in range(2):
                    for q in range(4):
                        filler_proj.append((c, ("q", c, hp, q)))
            filler_op = deque()

            def push_ops(b, qb):
                rb = b * 16 + qb
                for dc in range(4):
                    filler_op.append((rb, dc))

            op_credit = [0]

            def pop(n=1):
                for _ in range(n):
                    # op units are x-independent PE work: allow one at each
                    # proj-unit boundary (head packet q==0, so no proj psum
                    # accumulation is open -> no pj-rotation deadlock)
                    if (filler_proj and filler_op and op_credit[0] > 0
                            and filler_proj[0][1][-1] == 0):
                        op_credit[0] -= 1
                        rb, dc = filler_op.popleft()
                        op_unit(rb, dc)
                    elif filler_proj:
                        _, u = filler_proj.popleft()
                        if u[-1] == 0:
                            op_credit[0] = 1
                        kv_packet(u[1], u[2]) if u[0] == "kv" else \
                            q_packet(u[1], u[2], u[3])
                    elif filler_op:
                        rb, dc = filler_op.popleft()
                        op_unit(rb, dc)
                    else:
                        break

            def drain_proj_through(c):
                while filler_proj and filler_proj[0][0] <= c:
                    _, u = filler_proj.popleft()
                    kv_packet(u[1], u[2]) if u[0] == "kv" else \
                        q_packet(u[1], u[2], u[3])

            def attn_group(b, m):
                qb0 = 2 * m
                pvt = pspool.tile([128, 1024], F32, name="pv", tag="pv",
                                  bufs=1)
                # last group: shallow pend so the final norm chains (which
                # gate the last out-projections) start as early as possible
                depth = 1 if (b, m) == (1, 7) else 3
                pend = []
                for kb in range(qb0 + 2):
                    attn_iter(b, m, kb, pvt, pend, depth)
                    pop(1)
                while pend:
                    pk, pa, ppr = pend.pop(0)
                    emit_pv(b, qb0, pk, pa, ppr, pvt)
                    pop(2)

            # ---- main schedule ----
            drain_proj_through(0)          # chunk 0 blob
            nc.sync.dma_start(out=wot[0][:, :], in_=wo_d[0, :, :])
            nc.sync.dma_start(out=wot[1][:, :], in_=wo_d[1, :, :])
            for b in range(B):
                for m in range(8):
                    drain_proj_through(b * 4 + (2 * m + 1) // 4)
                    attn_group(b, m)
            while filler_proj or filler_op:
                pop(1)

            if dbg:
                nc.sync.dma_start(out=kT_dbg[:, :], in_=kT[0:64, :])
                nc.sync.dma_start(out=vO_dbg[:, :], in_=vOnes[:, :])
                nc.sync.dma_start(out=qT_dbg[:, :], in_=qT4[0:64, :])
                nc.sync.dma_start(out=aT_dbg[:, :], in_=attnT[:, :])

    nc.compile()
    return nc


# --------------------------------------------------------------------------
# host-side sharding / layout prep
# --------------------------------------------------------------------------

def _deint(h):
    """de-interleaved column indices for head h (64 cols: evens then odds)."""
    return h * HD + np.concatenate([np.arange(0, HD, 2), np.arange(1, HD, 2)])


def _prep_shared(x, freqs_cos, freqs_sin, mask):
    xT = np.ascontiguousarray(
        x.reshape(BS, D).T).astype(BF16)                   # [D, BS]
    crep = np.ascontiguousarray(freqs_cos.T).astype(BF16)  # [32, S]
    sT = freqs_sin.T                                       # [32, S]
    # sign pattern [+s, -s]: matches the DMA-free rotate-half muls that
    # read the raw projection with a 32-row partition shift
    ssig = np.concatenate([sT, -sT], axis=0).astype(BF16)  # [64, S]
    dm = np.exp(mask[0:128, 0:128]).T                      # [128k, 128q]
    dmask = np.tile(dm, (1, 4)).astype(BF16)               # [128, 512]
    return xT, crep, ssig, dmask


def _prep_core(c, wq, wk, wv, wo):
    heads = [4 * c + h for h in range(4)]
    # wq: [2, 128, 2048] (head-pair, kdim-within-tile, k-tile*128 + head col)
    qcols = np.concatenate([_deint(h) for h in heads])     # [256]
    wq_c = wq[:, qcols].reshape(16, 128, 2, 128).transpose(2, 1, 0, 3)
    wq_c = np.ascontiguousarray(wq_c.reshape(2, 128, 2048)).astype(BF16)
    # wkv: [128, 2048] = per k-tile 128 cols: [wk deint 64 | wv natural 64]
    kcols = _deint(c)
    vcols = c * HD + np.arange(HD)
    wkv = np.concatenate([wk[:, kcols], wv[:, vcols]], axis=1)  # [D, 128]
    wkv_c = wkv.reshape(16, 128, 128).transpose(1, 0, 2)
    wkv_c = np.ascontiguousarray(wkv_c.reshape(128, 2048)).astype(BF16)
    # wo: [2, 128, D] rows = heads 2t, 2t+1 natural hd
    worows = np.concatenate([h * HD + np.arange(HD) for h in heads])
    wo_c = np.ascontiguousarray(wo[worows, :].reshape(2, 128, D)).astype(BF16)
    return wq_c, wkv_c, wo_c


def _assemble(results):
    acc = np.zeros((BS, D), np.float32)
    for i in range(NCORES):
        acc += np.asarray(results[i]["out"], dtype=np.float32)
    return np.ascontiguousarray(acc.reshape(B, S, D))


LAST_RUN_INFO = {}


def kernel(x, freqs_cos, freqs_sin, mask, wq, wk, wv, wo, start_pos=0):
    from concourse.bass_utils import run_bass_kernel_spmd

    x = np.asarray(x, dtype=np.float32)
    freqs_cos = np.asarray(freqs_cos, dtype=np.float32)
    freqs_sin = np.asarray(freqs_sin, dtype=np.float32)
    mask = np.asarray(mask, dtype=np.float32)
    wq = np.asarray(wq, dtype=np.float32)
    wk = np.asarray(wk, dtype=np.float32)
    wv = np.asarray(wv, dtype=np.float32)
    wo = np.asarray(wo, dtype=np.float32)

    xT, crep, ssig, dmask = _prep_shared(x, freqs_cos, freqs_sin, mask)
    in_maps = []
    for c in range(NCORES):
        wq_c, wkv_c, wo_c = _prep_core(c, wq, wk, wv, wo)
        in_maps.append({
            "xT": xT, "wq": wq_c, "wkv": wkv_c, "wo": wo_c,
            "crep": crep, "ssig": ssig, "dmask": dmask,
        })

    nc = _build_nc()

    trace = bool(int(os.environ.get("KERNEL_TRACE", "0")))
    kwargs = {}
    if trace:
        _install_ntff_hook()
        import concourse.bass_utils as bass_utils
        bass_utils.upload_artifacts = lambda tmpdir: tmpdir
        import tempfile
        tmpdir = tempfile.mkdtemp(prefix="attn_trace_")
        kwargs = {"trace": True, "tmpdir": tmpdir}

    res = run_bass_kernel_spmd(nc, in_maps, core_ids=list(range(NCORES)),
                               **kwargs)
    LAST_RUN_INFO.clear()
    LAST_RUN_INFO.update({
        "exec_time_ns": res.exec_time_ns,
        "tmpdir": kwargs.get("tmpdir"),
        "res": res,
    })
    return _assemble(res.results)


def _install_ntff_hook():
    if "antenv.axon_hooks" not in sys.modules:
        import antenv

        mod = types.ModuleType("antenv.axon_hooks")
        mod._hook = None
        mod.set_axon_ntff_profile_hook = lambda h: setattr(mod, "_hook", h)
        mod.get_axon_ntff_profile_hook = lambda: mod._hook
        sys.modules["antenv.axon_hooks"] = mod
        antenv.axon_hooks = mod
    from trn_agent_boot.trn_boot import _ntff_profile_via_ctypes
    from antenv.axon_hooks import set_axon_ntff_profile_hook as _set

    _set(_ntff_profile_via_ctypes("/opt/axon/libaxon_pjrt.so"))
